# revision 42
# baseline (speedup 1.0000x reference)
"""Trainium2 Bass kernel for nn_MultiHeadAttention_41455024341166.

Reference computation (B=4, S=2048, M=2048, H=16, D=128, fp32):
    qkv = einsum('bsm,mthd->bsthd', x, Wqkv); q,k,v = qkv[:,:,0..2]
    q,k = rope_consecutive(q), rope_consecutive(k)
    ctx = causal_softmax(q @ k^T / sqrt(D)) @ v   (per b,h)
    out = ctx.reshape(B,S,H*D) @ Wo
Sharding: 8 cores = 4 batches x 2 head-groups (core c -> b=c//2, g=c%2,
heads [8g, 8g+8)). Head-parallel attention, pairwise ReduceScatter of the
output-projection partials.

v2 strategy (vs the fp32r baseline at 1.17ms): everything bf16, and the
whole working set stays resident in SBUF so phase B/C need no input DMA:
  - inputs are pre-cast to bf16 on the host; x^T (8MB), Wv (4MB) stream in
    once; qrot/krot (4MB each), v (4MB) and Wo (4MB) live in SBUF.
  - A_v first (x + wv + v resident = ~200KB/partition peak), then A_qk
    (wv pool closed; wblk + rope working set), writing RoPE'd q/k straight
    into resident tiles from the DVE — no DRAM roundtrip at all.
  - B: scoresT = krot-block stationary @ qrot-moving, exp fused into PSUM
    evacuation, causal diag via multiplicative mask; denominators via
    ones-vector matmuls; reciprocal now runs on [128,512] after a
    broadcast matmul (the [1,512] DVE reciprocal was 2.6us serial).
  - C: out partials accumulate against resident Wo, emitted per strip.
  - D: pairwise ReduceScatter per strip in bf16 (half the collective
    bytes); host upcasts y to fp32.
"""

import os
import sys
import types
import math

import numpy as np
import ml_dtypes

import concourse.bass as bass
import concourse.tile as tile
import concourse.mybir as mybir
from concourse.bass_utils import run_bass_kernel_spmd

F32 = mybir.dt.float32
BF16 = mybir.dt.bfloat16
NPBF = ml_dtypes.bfloat16

B, S, M, H, D = 4, 2048, 2048, 16, 128
HL = H // 2              # heads per core
HD = HL * D              # 1024
SCALE = 1.0 / math.sqrt(D)
MIN_WINDOW, MAX_WINDOW = 1.0, 10000.0

DEBUG = os.environ.get("MHA_KERNEL_DEBUG", "0") == "1"


# ---------------------------------------------------------------------------
# Workarounds for the trimmed walrus/axon stack in this container.
# ---------------------------------------------------------------------------

_WSPLIT_N = [0]


def _split_excess_waits(nc):
    """walrus here rejects instructions carrying more sync-waits than slots
    (1; EventSemaphore: 2). Hoist excess waits onto EventSemaphore carriers
    inserted before the offender on the same engine stream. Safe: Tile emits
    one linearized order where every wait's producer precedes its consumer."""
    for fn in nc.m.functions:
        for bb in fn.blocks:
            changed = False
            new_list = []
            for inst in bb.instructions:
                si = inst.sync_info
                waits = list(si.on_wait) if si is not None else []
                cap = 2 if isinstance(inst, mybir.InstEventSemaphore) else 1
                if len(waits) > cap:
                    keep, excess = waits[-cap:], waits[:-cap]
                    for i in range(0, len(excess), 2):
                        _WSPLIT_N[0] += 1
                        new_list.append(mybir.InstEventSemaphore(
                            name=f"wsplit-{_WSPLIT_N[0]}", ins=[], outs=[],
                            engine=inst.engine,
                            sync_info=mybir.SyncInfo(on_wait=excess[i:i + 2],
                                                     on_update=[])))
                    si.on_wait = keep
                    changed = True
                new_list.append(inst)
            if changed:
                bb.instructions = new_list


def _register_ntff_hook():
    """antenv.axon_hooks is absent in this image, so boot skipped registering
    the NTFF profiling hook; recreate it so trace=True works."""
    if "antenv.axon_hooks" in sys.modules:
        return
    try:
        import antenv as _antenv
        m = types.ModuleType("antenv.axon_hooks")
        m._hook = None
        m.set_axon_ntff_profile_hook = lambda h, _m=m: setattr(_m, "_hook", h)
        m.get_axon_ntff_profile_hook = lambda _m=m: _m._hook
        sys.modules["antenv.axon_hooks"] = m
        _antenv.axon_hooks = m
        from trn_agent_boot.trn_boot import _ntff_profile_via_ctypes
        m.set_axon_ntff_profile_hook(
            _ntff_profile_via_ctypes('/opt/axon/libaxon_pjrt.so'))
    except Exception:
        pass


_register_ntff_hook()


# ---------------------------------------------------------------------------
# Kernel builder (per-core SPMD program)
# ---------------------------------------------------------------------------

def build_kernel():
    nc = bass.Bass("TRN2", target_bir_lowering=False, num_devices=8)

    xt = nc.dram_tensor("xt", [M, S], BF16, kind="ExternalInput")       # x[b].T
    wq = nc.dram_tensor("wq", [M, HD], BF16, kind="ExternalInput")
    wk = nc.dram_tensor("wk", [M, HD], BF16, kind="ExternalInput")
    wv = nc.dram_tensor("wv", [M, HD], BF16, kind="ExternalInput")
    wo = nc.dram_tensor("wo", [HD, M], BF16, kind="ExternalInput")
    cosT = nc.dram_tensor("cosT", [D, S], BF16, kind="ExternalInput")
    sinT = nc.dram_tensor("sinT", [D, S], BF16, kind="ExternalInput")   # sign-folded
    pmat = nc.dram_tensor("pmat", [D, D], BF16, kind="ExternalInput")   # pair swap
    mask128 = nc.dram_tensor("mask128", [128, 128], BF16, kind="ExternalInput")
    ident128 = nc.dram_tensor("ident128", [128, 128], BF16, kind="ExternalInput")
    # RS quarters: y[t] = out[b, t*512 + half*256 : +256, :] for this core's half
    y = nc.dram_tensor("y", [4, 256, M], BF16, kind="ExternalOutput")

    dbg = {}
    if DEBUG:
        dbg["qrot"] = nc.dram_tensor("dbg_qrot", [HD, S], BF16, kind="ExternalOutput")
        dbg["krot"] = nc.dram_tensor("dbg_krot", [HD, S], BF16, kind="ExternalOutput")
        dbg["v"] = nc.dram_tensor("dbg_v", [S, HD], BF16, kind="ExternalOutput")
        dbg["ctxT"] = nc.dram_tensor("dbg_ctxT", [HD, S], BF16, kind="ExternalOutput")
        dbg["outp"] = nc.dram_tensor("dbg_outp", [S, M], BF16, kind="ExternalOutput")

    def blk(dram_full, a):
        """Row-block a (rows [a*128,(a+1)*128)) of a [R, C] DRAM tensor as a
        [128, C] DMA source."""
        return dram_full.rearrange("(a p) c -> p a c", p=128)[:, a, :]

    with nc.allow_low_precision(reason="bf16 matmul kernel"), \
         tile.TileContext(nc) as tc:
        with tc.tile_pool(name="dram", bufs=1, space="DRAM") as dram, \
             tc.tile_pool(name="res", bufs=1) as res:
            # -------- resident SBUF tensors (alive A..C) --------
            qrot_sb = [res.tile([128, S], BF16, name=f"qrot{h}") for h in range(HL)]
            krot_sb = [res.tile([128, S], BF16, name=f"krot{h}") for h in range(HL)]
            # v with a ones column per head: [v_h | 1] blocks of 129 so the
            # flipped PV matmul emits softmax denominators as output col 128
            v_sb = [res.tile([128, HL * (D + 1)], BF16, name=f"vsb{sb}")
                    for sb in range(16)]
            for sb in range(16):
                nc.gpsimd.memset(
                    v_sb[sb][:].rearrange("p (h c) -> p h c", c=D + 1)
                    [:, :, D:D + 1], 1.0)
            mask_sb = res.tile([128, 128], BF16)
            nc.gpsimd.dma_start(mask_sb[:], mask128[:])
            p_sb = res.tile([128, 128], BF16)
            nc.gpsimd.dma_start(p_sb[:], pmat[:])
            ident_sb = res.tile([128, 128], BF16)
            nc.gpsimd.dma_start(ident_sb[:], ident128[:])

            # ======== Phase A: projections off one resident xT ========
            with tc.tile_pool(name="ax", bufs=1) as xp:
                xts = []

                # ---- A_v: v = x @ Wv into resident [s, hd] tiles ----
                # mt-outer over groups of 4 s-blocks (8 PSUM banks) so the PE
                # has 8 matmuls of work per arriving x row-block instead of 2,
                # hiding most of the initial x/wv DMA wall.
                with nc.named_scope("A_v"):
                    with (
                        tc.tile_pool(name="awv", bufs=1) as wvp,
                        tc.tile_pool(name="avps", bufs=1, space="PSUM") as psvp,
                    ):
                        wv_sb = []
                        for mt in range(16):
                            # wv on the scalar queue, x on sync: two issue
                            # rings in parallel during the cold start. x lands
                            # in quarter-column pieces in consumption order so
                            # the first matmul waits on ~384KB, not 768KB.
                            wvt = wvp.tile([128, HD], BF16, name=f"wvt{mt}")
                            nc.scalar.dma_start(wvt[:], blk(wv, mt))
                            xti = xp.tile([128, S], BF16, name=f"xt{mt}")
                            nc.sync.dma_start(xti[:, 0:512], blk(xt, mt)[:, 0:512])
                            wv_sb.append(wvt)
                            xts.append(xti)
                        for q in range(1, 4):
                            for mt in range(16):
                                nc.sync.dma_start(
                                    xts[mt][:, q * 512:(q + 1) * 512],
                                    blk(xt, mt)[:, q * 512:(q + 1) * 512])
                        for sbg in range(4):
                            pss = {}
                            for sbl in range(4):
                                for ds in range(2):
                                    pss[(sbl, ds)] = psvp.tile(
                                        [128, 512], F32,
                                        name=f"psv{sbg}{sbl}{ds}",
                                        tag=f"psv{sbl}{ds}")
                            for mt in range(16):
                                for sbl in range(4):
                                    sb = sbg * 4 + sbl
                                    for ds in range(2):
                                        nc.tensor.matmul(
                                            pss[(sbl, ds)][:],
                                            xts[mt][:, sb * 128:(sb + 1) * 128],
                                            wv_sb[mt][:, ds * 512:(ds + 1) * 512],
                                            start=(mt == 0), stop=(mt == 15),
                                            skip_group_check=True)
                            for sbl in range(4):
                                sb = sbg * 4 + sbl
                                for ds in range(2):
                                    # one wide strided copy per PSUM bank into
                                    # the 129-strided layout (ACT/DVE split)
                                    dst = v_sb[sb][:].rearrange(
                                        "p (h c) -> p h c", c=D + 1)[
                                        :, ds * 4:(ds + 1) * 4, 0:D]
                                    src = pss[(sbl, ds)][:].rearrange(
                                        "p (h c) -> p h c", c=D)
                                    if ds == 0:
                                        nc.scalar.copy(dst, src)
                                    else:
                                        nc.vector.tensor_copy(dst, src)
                        if DEBUG:
                            for sb in range(16):
                                nc.gpsimd.dma_start(
                                    dbg["v"][sb * 128:(sb + 1) * 128, :],
                                    v_sb[sb][:].rearrange(
                                        "p (h c) -> p h c", c=D + 1)[:, :, 0:D])

                # ---- A_qk: qT,kT + RoPE into resident [d, s] tiles ----
                with nc.named_scope("A_qk"):
                    with (
                        tc.tile_pool(name="atab", bufs=1) as tabp,
                        tc.tile_pool(name="aw", bufs=3) as wp,
                        tc.tile_pool(name="aps", bufs=3, space="PSUM") as psp,
                        tc.tile_pool(name="aps2", bufs=2, space="PSUM") as psp2,
                        tc.tile_pool(name="at", bufs=3) as tp,
                    ):
                        cos_sb = tabp.tile([128, S], BF16)
                        nc.gpsimd.dma_start(cos_sb[:], cosT[:])
                        sin_sb = tabp.tile([128, S], BF16)
                        nc.gpsimd.dma_start(sin_sb[:], sinT[:])

                        groups = [(h, qk) for h in range(HL) for qk in range(2)]
                        wts = {0: wq, 1: wk}
                        wblks = {}

                        def prefetch(gi):
                            if gi >= len(groups):
                                return
                            h, qk = groups[gi]
                            wb = wp.tile([128, 16 * 128], BF16,
                                         name=f"wblk{h}{qk}", tag="wblk")
                            src = wts[qk].rearrange("(a p) c -> p a c", p=128)
                            nc.sync.dma_start(
                                wb[:].rearrange("p (a c) -> p a c", c=128),
                                src[:, :, h * 128:(h + 1) * 128])
                            wblks[gi] = wb

                        prefetch(0)
                        prefetch(1)

                        # deferred RoPE perm matmuls: (ps2_dst, q_src)
                        pperm = []

                        def flush_perm():
                            if pperm:
                                dst, src = pperm.pop(0)
                                nc.tensor.matmul(dst[:], p_sb[:], src[:],
                                                 start=True, stop=True)

                        rope_tail = []

                        def emit_rope(h, qk, t, q_sb, ps2):
                            outt = qrot_sb if qk == 0 else krot_sb
                            t1 = tp.tile([128, 512], F32,
                                         name=f"t1{h}{qk}{t}", tag="t1")
                            nc.vector.tensor_mul(t1[:], q_sb[:],
                                                 cos_sb[:, t * 512:(t + 1) * 512])
                            t2 = tp.tile([128, 512], F32,
                                         name=f"t2{h}{qk}{t}", tag="t2")
                            nc.vector.tensor_mul(t2[:], ps2[:],
                                                 sin_sb[:, t * 512:(t + 1) * 512])
                            nc.vector.tensor_add(
                                outt[h][:, t * 512:(t + 1) * 512], t1[:], t2[:])

                        for gi, (h, qk) in enumerate(groups):
                            prefetch(gi + 2)
                            wblk = wblks.pop(gi)
                            for t in range(4):
                                ps = psp.tile([128, 512], F32,
                                              name=f"psq{h}{qk}{t}", tag="psq")
                                for mt in range(16):
                                    nc.tensor.matmul(
                                        ps[:],
                                        wblk[:, mt * 128:(mt + 1) * 128],
                                        xts[mt][:, t * 512:(t + 1) * 512],
                                        start=(mt == 0), stop=(mt == 15))
                                q_sb = tp.tile([128, 512], BF16,
                                               name=f"q{h}{qk}{t}", tag="q")
                                nc.scalar.copy(q_sb[:], ps[:])
                                ps2 = psp2.tile([128, 512], F32,
                                                name=f"psw{h}{qk}{t}", tag="psw")
                                pperm.append((ps2, q_sb))
                                if len(pperm) > 1:
                                    flush_perm()
                                while rope_tail:
                                    emit_rope(*rope_tail.pop(0))
                                rope_tail.append((h, qk, t, q_sb, ps2))
                        flush_perm()
                        while rope_tail:
                            emit_rope(*rope_tail.pop(0))

                        if DEBUG:
                            for h in range(HL):
                                nc.sync.dma_start(
                                    dbg["qrot"][h * 128:(h + 1) * 128, :],
                                    qrot_sb[h][:])
                                nc.sync.dma_start(
                                    dbg["krot"][h * 128:(h + 1) * 128, :],
                                    krot_sb[h][:])

            # ======== Phase B+C+D: attention (query-strip outer), output ====
            with (
                tc.tile_pool(name="bwo", bufs=1) as wop,
                tc.tile_pool(name="bctx", bufs=1) as cxp,
                tc.tile_pool(name="bex", bufs=8) as exp_,
                tc.tile_pool(name="bcn", bufs=2) as cnp,
                tc.tile_pool(name="brc", bufs=2) as rcp,
                tc.tile_pool(name="bps", bufs=3, space="PSUM") as pssp,
                tc.tile_pool(name="bpa", bufs=1, space="PSUM") as bpap,
                tc.tile_pool(name="btr", bufs=1, space="PSUM") as btrp,
                tc.tile_pool(name="co", bufs=4) as cop,
            ):
                wo_sb = []
                for ht in range(HL):
                    wot = wop.tile([128, M], BF16, name=f"wot{ht}")
                    nc.sync.dma_start(wot[:], blk(wo, ht))
                    wo_sb.append(wot)
                ctx_sb = [cxp.tile([128, S], BF16, name=f"ctx{h}")
                          for h in range(HL)]

                outp_t = [dram.tile([512, M], BF16, name=f"outp{i}")
                          for i in range(3)]
                rs_t = [dram.tile([256, M], BF16, name=f"rst{i}")
                        for i in range(3)]
                # final strip's RS in 2 pieces: [1024, 1024] output cols
                p3w = [1024, 1024]
                outp3 = [dram.tile([512, w], BF16, name=f"outp3{i}")
                         for i, w in enumerate(p3w)]
                rs3 = [dram.tile([256, w], BF16, name=f"rst3{i}")
                       for i, w in enumerate(p3w)]
                ms2piece = {0: (0, 0), 1: (0, 512), 2: (1, 0), 3: (1, 512)}

                # strip-deferred transpose state: (ctxn, h, t). The normalized
                # ctx chunks are produced in [i, d] layout; a PE transpose puts
                # them into the resident [d, s] ctx tiles that C consumes.
                pending = []

                def flush_pending():
                    if not pending:
                        return
                    cnn_, hp_, tp2_ = pending.pop()
                    ptr = btrp.tile([128, 512], BF16,
                                    name=f"ptr{hp_}{tp2_}", tag="ptr")
                    for c in range(4):
                        nc.tensor.transpose(
                            ptr[:, c * 128:(c + 1) * 128],
                            cnn_[:, c * 128:(c + 1) * 128], ident_sb[:])
                    # DVE, not ACT: post-flip the Scalar engine (exp stream)
                    # is the B-phase wall; DVE has slack here
                    nc.vector.tensor_copy(
                        ctx_sb[hp_][:, tp2_ * 512:(tp2_ + 1) * 512], ptr[:])

                def trigger_rs(tt):
                    nc.gpsimd.collective_compute(
                        "ReduceScatter", mybir.AluOpType.add,
                        replica_groups=[[0, 1], [2, 3], [4, 5], [6, 7]],
                        ins=[outp_t[tt][:]], outs=[rs_t[tt][:]])

                with nc.named_scope("B_attn"):
                    for t in range(4):
                        # mid-kernel RS triggers deferred two strips: the
                        # collective's DMA traffic then overlaps pure attention
                        # compute and never contends with C_out's output DMAs
                        # (results are only consumed by the y writes at the
                        # kernel end).
                        if t == 2:
                            trigger_rs(0)
                        elif t == 3:
                            trigger_rs(1)
                            trigger_rs(2)
                        njt = 4 * t + 4
                        # cross-head pipelined emission: score/exp fronts run
                        # LAG blocks ahead of the pv/sum backs so the PE never
                        # drains at head boundaries (t=0 heads are only 4
                        # blocks long).
                        LAG = 6
                        state = {}   # h -> (pc, pm)
                        backlog = []

                        def emit_back(h, jt, ex, cut):
                            # flipped PV: prob chunk stationary (128-col bf16
                            # -> FWL), ones-augmented v moving (129 rows).
                            # Output [i, d | denom] accumulates per i-chunk;
                            # chunk c participates in blocks jt <= 4t+c.
                            pacc, rec, cnn = state[h]
                            vaug = v_sb[jt][:, h * (D + 1):
                                            (h + 1) * (D + 1)]
                            for c in range(cut // 128, 4):
                                nc.tensor.matmul(
                                    pacc[c][:],
                                    ex[:, c * 128:(c + 1) * 128],
                                    vaug,
                                    start=(jt == 0), stop=(jt == 4 * t + c),
                                    skip_group_check=True)
                                if jt == 4 * t + c:
                                    nc.vector.reciprocal(
                                        rec[:, c:c + 1], pacc[c][:, D:D + 1])
                                    nc.vector.tensor_scalar_mul(
                                        cnn[:, c * 128:(c + 1) * 128],
                                        pacc[c][:, 0:D], rec[:, c:c + 1])
                            if jt == 1:
                                flush_pending()
                            if jt == njt - 1:
                                del state[h]
                                pending.append((cnn, h, t))

                        for h in range(HL):
                            qr = qrot_sb[h][:, t * 512:(t + 1) * 512]
                            state[h] = (
                                [bpap.tile([128, D + 1], F32,
                                           name=f"pa{h}{t}{c}", tag=f"pa{c}")
                                 for c in range(4)],
                                rcp.tile([128, 4], F32,
                                         name=f"rc{h}{t}", tag="rc"),
                                cnp.tile([128, 512], BF16,
                                         name=f"cn{h}{t}", tag="cn"))
                            for jt in range(njt):
                                # scoresT block + exp into SBUF (+ diag mask)
                                cut = 128 * (jt - 4 * t) if jt >= 4 * t else 0
                                pss = pssp.tile([128, 512], F32,
                                                name=f"pss{h}{t}{jt}", tag="pss")
                                nc.tensor.matmul(
                                    pss[:, cut:512],
                                    krot_sb[h][:, jt * 128:(jt + 1) * 128],
                                    qr[:, cut:512], start=True, stop=True,
                                    skip_group_check=True)
                                ex = exp_.tile([128, 512], BF16,
                                               name=f"ex{h}{t}{jt}", tag="ex")
                                nc.scalar.activation(
                                    ex[:, cut:512], pss[:, cut:512],
                                    mybir.ActivationFunctionType.Exp,
                                    scale=SCALE)
                                if jt >= 4 * t:
                                    # SBUF-only op: run on the otherwise-idle
                                    # GpSimd so the DVE keeps up at t=0
                                    nc.gpsimd.tensor_mul(
                                        ex[:, cut:cut + 128],
                                        ex[:, cut:cut + 128], mask_sb[:])
                                backlog.append((h, jt, ex, cut))
                                if len(backlog) > LAG:
                                    emit_back(*backlog.pop(0))
                        while backlog:
                            emit_back(*backlog.pop(0))

                        # ---- output row chunk for this strip + ReduceScatter
                        flush_pending()
                        with nc.named_scope(f"C_out{t}"):
                            for ms in range(4):
                                for sbl in range(4):
                                    sb = 4 * t + sbl
                                    po = pssp.tile([128, 512], F32,
                                                   name=f"po{t}{sbl}{ms}",
                                                   tag="pss")
                                    for ht in range(HL):
                                        nc.tensor.matmul(
                                            po[:],
                                            ctx_sb[ht][:, sb * 128:(sb + 1) * 128],
                                            wo_sb[ht][:, ms * 512:(ms + 1) * 512],
                                            start=(ht == 0), stop=(ht == HL - 1))
                                    ot = cop.tile([128, 512], BF16,
                                                  name=f"ot{t}{sbl}{ms}", tag="ot")
                                    # DVE only: routing any ot copies through
                                    # the Scalar FIFO couples the output-DMA
                                    # backlog into the next strip's exps
                                    nc.vector.tensor_copy(ot[:], po[:])
                                    if t == 3:
                                        pc3, co3 = ms2piece[ms]
                                        dst = outp3[pc3][
                                            sbl * 128:(sbl + 1) * 128,
                                            co3:co3 + 512]
                                    else:
                                        dst = outp_t[t][
                                            sbl * 128:(sbl + 1) * 128,
                                            ms * 512:(ms + 1) * 512]
                                    nc.sync.dma_start(dst, ot[:])
                                    if DEBUG:
                                        nc.sync.dma_start(
                                            dbg["outp"][sb * 128:(sb + 1) * 128,
                                                        ms * 512:(ms + 1) * 512],
                                            ot[:])
                                if t == 3 and ms in (1, 3):
                                    pc3 = ms2piece[ms][0]
                                    nc.gpsimd.collective_compute(
                                        "ReduceScatter", mybir.AluOpType.add,
                                        replica_groups=[[0, 1], [2, 3], [4, 5], [6, 7]],
                                        ins=[outp3[pc3][:]], outs=[rs3[pc3][:]])
                            if t == 3:
                                for tt in range(3):
                                    nc.sync.dma_start(y[tt], rs_t[tt][:])
                                co3 = 0
                                for pc3, w in enumerate(p3w):
                                    nc.sync.dma_start(
                                        y[3][:, co3:co3 + w], rs3[pc3][:])
                                    co3 += w

                if DEBUG:
                    for h in range(HL):
                        nc.sync.dma_start(dbg["ctxT"][h * 128:(h + 1) * 128, :],
                                          ctx_sb[h][:])

    _split_excess_waits(nc)
    return nc


# ---------------------------------------------------------------------------
# Host-side input prep / sharding
# ---------------------------------------------------------------------------

def _rope_tables():
    half = D // 2
    fraction = 2.0 * np.arange(half, dtype=np.float64) / D
    ts = MIN_WINDOW * (MAX_WINDOW / MIN_WINDOW) ** fraction
    ts = np.repeat(ts, 2)                              # [D]
    pos = np.arange(S, dtype=np.float64)
    sinusoid = pos[None, :] / ts[:, None]              # [D, S]
    cos = np.cos(sinusoid).astype(NPBF)
    sign = np.where(np.arange(D) % 2 == 1, 1.0, -1.0)
    sin = (np.sin(sinusoid) * sign[:, None]).astype(NPBF)
    return cos, sin


def _mask128():
    jj = np.arange(128)[:, None]
    ii = np.arange(128)[None, :]
    return (jj <= ii).astype(NPBF)


def _pmat():
    p = np.zeros((D, D), dtype=NPBF)
    idx = np.arange(D)
    p[idx, idx ^ 1] = 1.0
    return p


_CACHED = {}


def kernel(x, Wqkv, Wo):
    x = np.asarray(x, dtype=np.float32)
    Wqkv = np.asarray(Wqkv, dtype=np.float32)
    Wo = np.asarray(Wo, dtype=np.float32)

    cos, sin = _rope_tables()
    m128 = _mask128()
    pm = _pmat()

    in_maps = []
    for c in range(8):
        b, g = c // 2, c % 2
        hs = slice(g * HL, (g + 1) * HL)
        in_maps.append({
            "xt": np.ascontiguousarray(x[b].T).astype(NPBF),
            "wq": np.ascontiguousarray(Wqkv[:, 0, hs, :].reshape(M, HD)).astype(NPBF),
            "wk": np.ascontiguousarray(Wqkv[:, 1, hs, :].reshape(M, HD)).astype(NPBF),
            "wv": np.ascontiguousarray(Wqkv[:, 2, hs, :].reshape(M, HD)).astype(NPBF),
            "wo": np.ascontiguousarray(Wo[g * HD:(g + 1) * HD, :]).astype(NPBF),
            "cosT": cos, "sinT": sin, "pmat": pm, "mask128": m128,
            "ident128": np.eye(128, dtype=NPBF),
        })

    if "nc" not in _CACHED:
        _CACHED["nc"] = build_kernel()
    nc = _CACHED["nc"]

    res = run_bass_kernel_spmd(nc, in_maps, core_ids=list(range(8)),
                               trace=os.environ.get("MHA_KERNEL_TRACE", "0") == "1")
    _CACHED["last_results"] = res

    out = np.empty((B, S, M), dtype=np.float32)
    for b in range(B):
        for half, r in ((0, res.results[2 * b]["y"]),
                        (256, res.results[2 * b + 1]["y"])):
            for t in range(4):
                out[b, t * 512 + half: t * 512 + half + 256] = \
                    np.asarray(r[t]).astype(np.float32)
    return out


if __name__ == "__main__":
    rng = np.random.default_rng(0)
    x = rng.standard_normal((B, S, M), dtype=np.float32)
    Wqkv = (rng.standard_normal((M, 3, H, D), dtype=np.float32) / math.sqrt(M)).astype(np.float32)
    Wo = (rng.standard_normal((H * D, M), dtype=np.float32) / math.sqrt(H * D)).astype(np.float32)
    out = kernel(x=x, Wqkv=Wqkv, Wo=Wo)
    print("kernel ran, out shape", out.shape, "mean", float(np.abs(out).mean()))
